# revision 28
# baseline (speedup 1.0000x reference)
"""Trainium2 Bass kernel for soft decision-tree histogram binning.

Computes out[b, j] = prod_f softmax((x[b,f]*W + b_f)/T)[digit_f(j)]
for x (4096, 7), cutpoints (7, 3) -> out (4096, 4**7=16384) float32.

Strategy (data-parallel over batch, 8 cores x 512 rows, 4 tiles of 128):
  - mixed-precision output: chunks 0-1 of each tile row go out as fp16
    (produced by DVE in its 4x mode, 330 ns/chunk), chunks 2-15 as fp8e4
    (DVE 2x mode 662 ns/chunk, ACT 1131 ns/chunk; fp8 has no DVE
    packing on cayman). fp8 halves the HBM drain that bounded the
    all-fp16 kernel (59.2us). Unnormalized kron factors lie in (0,1]
    with the per-row max exactly 1.0 (exp(h-max)), which fp8e4 (RTNE,
    matches ml_dtypes e4m3) represents exactly: measured end-to-end rel
    err 8.0e-3 vs the 2e-2 gate.
  - softmax denominators are NOT applied on device: zp = prod_f sum_d e
    goes out as a tiny side output; the host folds 1/zp into the upcast.
  - h-chain and z-products are fused across all 4 tiles ([P,112] ops);
    t4 is fp16 so the t5 builders hit DVE 4x.
  - output DMAs: few and large (2-5 chunks each), all on the sync HWDGE
    ring, emitted per tile in expected completion order [fp16,
    DVE-half, ACT-half, DVE-half, ACT-half] (the sync FIFO dispatches
    strictly in emission order). The last tile ends with single-chunk
    groups so the final drain lump is small.
  - the four cascades (one fused 4D-AP op per level for t2/t3/t4/sc16
    across all tiles + 16 t5 builders) are built entirely upfront, so
    from ~16us both engine queues are pure chunk production with no
    mid-run lulls to starve the DMA stream (-1us vs per-tile cascades).
  - measured structural floor ~45-47us: ACT ends ~41.4 (busy 28.9),
    DVE ~40.8 (busy 30.7, 98% duty), then the last lump drains at the
    HBM-stack-shared DMA ceiling (~0.39 MiB/us/core; all 8 cores burst
    in lockstep) plus ~1.5us completion receipt + lean exit. Front is
    input-sem gated: input data lands ~6us but its completion sem fires
    ~9.2us (HBM receipt round-trip); a DVE probe op confirmed the
    engine itself is awake at ~7us.
  Tried and measurably worse:
  - per-chunk DMAs in completion order (+15us: per-DMA sync-sequencer
    overhead dominates),
  - ACT- or GpSimd-dispatched DMAs (the dispatching engine's compute
    stream stalls on ring backpressure; GpSimd SWDGE DIRECT2D also
    costs 6-8us descriptor gen per block),
  - GPSIMD as third fp8 producer (15-22us per chunk software dtype
    conversion + SBUF port contention stretches 2-port DVE ops ~3x),
  - more fp16 chunks (C16=5: bytes exceed the shared-stack drain
    ceiling, backpressure stalls both producers),
  - PE diag-matmul producer: dead end, DMA has no PSUM route on trn2
    so evacuation would cost the same DVE/ACT elem throughput anyway.
"""

import numpy as np

B = 4096
F = 7
D1 = 4  # D+1 bins per feature
OUT = D1**F  # 16384
NCORES = 8
ROWS = B // NCORES  # 512
P = 128
NTILES = ROWS // P  # 4
TEMPERATURE = 0.1

NX = NTILES * F  # 28 x columns
XWC = NX + D1 + NTILES * F * D1  # x (28) | W/T (4) | b/T replicated (112)

N16 = 2  # chunks 0..N16-1 are fp16 (DVE 4x); rest fp8
C16 = N16 * 1024
C8 = OUT - C16

# per-tile-parity fp8 chunk assignment
DVE8 = {0: [2, 3, 4, 5, 6, 7, 8, 9], 1: [2, 3, 4, 5, 6, 7, 8, 9]}
ACT8 = {0: [10, 11, 12, 13, 14, 15], 1: [10, 11, 12, 13, 14, 15]}

_cache = {}


def _build_bass():
    import concourse.bacc as bacc
    import concourse.tile as tile
    from concourse import mybir

    f32 = mybir.dt.float32
    f16 = mybir.dt.float16
    f8 = mybir.dt.float8e4
    Alu = mybir.AluOpType
    Act = mybir.ActivationFunctionType
    AX = mybir.AxisListType.X

    from concourse.vector_clock import ScopedClock

    class LeanTileContext(tile.TileContext):
        """TileContext with a minimal kernel exit: keep the sync-engine
        drain that waits for all outstanding work, skip the two
        all-engine barriers and the semaphore recycle loop."""

        def _drain_and_barrier(self, tick_clock, wait_clock):
            drain_inst = self.nc.sync.drain()
            wait_clock.add_sem_waits(
                drain_inst.ins, ScopedClock({None: tick_clock.global_clock})
            )
            popped = self.nc._tile_sem_poison_stack.pop()
            assert popped is self._sem_poison

    nc = bacc.Bacc("TRN2", target_bir_lowering=False, debug=False)

    xw_d = nc.dram_tensor("xw", [P, XWC], f32, kind="ExternalInput").ap()
    o16_d = nc.dram_tensor("o16", [ROWS, C16], f16, kind="ExternalOutput").ap()
    o8_d = nc.dram_tensor("o8", [ROWS, C8], f8, kind="ExternalOutput").ap()
    z_d = nc.dram_tensor("zp", [P, NTILES], f32, kind="ExternalOutput").ap()

    with LeanTileContext(nc) as tc:
        with (
            tc.tile_pool(name="const", bufs=1) as cpool,
            tc.tile_pool(name="small", bufs=3) as sp,
            tc.tile_pool(name="mid", bufs=5) as mp,
            tc.tile_pool(name="blk", bufs=8) as blkp,
            tc.tile_pool(name="blka", bufs=6) as bap,
        ):
            xw = cpool.tile([P, XWC], f32)
            nc.sync.dma_start(out=xw, in_=xw_d)
            w4 = xw[:, NX : NX + D1][:, None, :].broadcast_to((P, NX, D1))
            ball = xw[:, NX + D1 :].rearrange("p (tf d) -> p tf d", d=D1)
            zbuf = cpool.tile([P, NTILES], f32)

            # fused h-chain over all 4 tiles: h[(t f), d] = x*(W/T) + b/T
            h = cpool.tile([P, NX * D1], f32)
            h3 = h.rearrange("p (tf d) -> p tf d", d=D1)
            xb = xw[:, 0:NX][:, :, None].broadcast_to((P, NX, D1))
            nc.vector.tensor_tensor(out=h3, in0=xb, in1=w4, op=Alu.mult)
            nc.vector.tensor_tensor(out=h3, in0=h3, in1=ball, op=Alu.add)
            m28 = cpool.tile([P, NX], f32)
            nc.vector.tensor_reduce(out=m28, in_=h3, axis=AX, op=Alu.max)
            mb = m28[:, :, None].broadcast_to((P, NX, D1))
            nc.vector.tensor_tensor(out=h3, in0=h3, in1=mb, op=Alu.subtract)
            e = cpool.tile([P, NX * D1], f32)
            nc.scalar.activation(out=e, in_=h, func=Act.Exp, scale=1.0)

            # tile 0's cascade goes first as small per-tile ops so ACT's
            # first chunk can start ~0.8us earlier; tiles 1-3 then use one
            # fused 4D-AP op per level (t2/t3/t4/sc16), amortizing DVE
            # op-init overhead. NT3 = tiles 1..3.
            NT3 = NTILES - 1
            e0 = e[:, 0:28]
            t2_0 = sp.tile([P, 16], f32, tag="t2")
            nc.vector.tensor_tensor(
                out=t2_0.rearrange("p (a b) -> p a b", b=D1),
                in0=e0[:, 20:24, None].broadcast_to((P, D1, D1)),
                in1=e0[:, None, 24:28].broadcast_to((P, D1, D1)),
                op=Alu.mult,
            )
            t3_0 = sp.tile([P, 64], f32, tag="t3")
            nc.vector.tensor_tensor(
                out=t3_0.rearrange("p (a b) -> p a b", b=16),
                in0=e0[:, 16:20, None].broadcast_to((P, D1, 16)),
                in1=t2_0[:, None, :].broadcast_to((P, D1, 16)),
                op=Alu.mult,
            )
            t4_0 = sp.tile([P, 256], f16, tag="t4")
            nc.vector.tensor_tensor(
                out=t4_0.rearrange("p (a b) -> p a b", b=64),
                in0=e0[:, 12:16, None].broadcast_to((P, D1, 64)),
                in1=t3_0[:, None, :].broadcast_to((P, D1, 64)),
                op=Alu.mult,
            )
            sc_0 = sp.tile([P, 16], f32, tag="sc16")
            nc.vector.tensor_tensor(
                out=sc_0.rearrange("p (a b) -> p a b", b=D1),
                in0=e0[:, 4:8, None].broadcast_to((P, D1, D1)),
                in1=e0[:, None, 0:4].broadcast_to((P, D1, D1)),
                op=Alu.mult,
            )
            t5_0 = mp.tile([P, 1024], f16, tag="t5")
            for d in range(D1):
                nc.vector.tensor_scalar_mul(
                    out=t5_0[:, d * 256 : (d + 1) * 256],
                    in0=t4_0,
                    scalar1=e0[:, 8 + d : 9 + d],
                )

            et4 = e[:, 28:].rearrange("p (t c) -> p t c", c=28)  # [P, 3, 28]
            t2a = cpool.tile([P, NT3 * 16], f32)
            nc.vector.tensor_tensor(
                out=t2a.rearrange("p (t a b) -> p t a b", a=D1, b=D1),
                in0=et4[:, :, 20:24][:, :, :, None].broadcast_to(
                    (P, NT3, D1, D1)
                ),
                in1=et4[:, :, 24:28][:, :, None, :].broadcast_to(
                    (P, NT3, D1, D1)
                ),
                op=Alu.mult,
            )
            t3a = cpool.tile([P, NT3 * 64], f32)
            nc.vector.tensor_tensor(
                out=t3a.rearrange("p (t a b) -> p t a b", a=D1, b=16),
                in0=et4[:, :, 16:20][:, :, :, None].broadcast_to(
                    (P, NT3, D1, 16)
                ),
                in1=t2a.rearrange("p (t b) -> p t b", b=16)[
                    :, :, None, :
                ].broadcast_to((P, NT3, D1, 16)),
                op=Alu.mult,
            )
            t4a = cpool.tile([P, NT3 * 256], f16)
            nc.vector.tensor_tensor(
                out=t4a.rearrange("p (t a b) -> p t a b", a=D1, b=64),
                in0=et4[:, :, 12:16][:, :, :, None].broadcast_to(
                    (P, NT3, D1, 64)
                ),
                in1=t3a.rearrange("p (t b) -> p t b", b=64)[
                    :, :, None, :
                ].broadcast_to((P, NT3, D1, 64)),
                op=Alu.mult,
            )
            sca = cpool.tile([P, NT3 * 16], f32)
            nc.vector.tensor_tensor(
                out=sca.rearrange("p (t a b) -> p t a b", a=D1, b=D1),
                in0=et4[:, :, 4:8][:, :, :, None].broadcast_to(
                    (P, NT3, D1, D1)
                ),
                in1=et4[:, :, 0:4][:, :, None, :].broadcast_to(
                    (P, NT3, D1, D1)
                ),
                op=Alu.mult,
            )

            def casc_b(t):
                """Cascade second half: the four t5 builders (DVE 4x)."""
                et = e[:, t * 28 : (t + 1) * 28]
                t4 = t4a[:, (t - 1) * 256 : t * 256]
                t5 = mp.tile([P, 1024], f16, tag="t5")
                for d in range(D1):
                    nc.vector.tensor_scalar_mul(
                        out=t5[:, d * 256 : (d + 1) * 256],
                        in0=t4,
                        scalar1=et[:, 8 + d : 9 + d],
                    )
                return t5

            def cascade(t):
                if t == 0:
                    return t5_0, sc_0
                return casc_b(t), sca[:, (t - 1) * 16 : t * 16]

            def scol(sc16, c):
                d0, d1 = c // D1, c % D1
                return sc16[:, d1 * D1 + d0 : d1 * D1 + d0 + 1]

            def grp16(t, t5, sc16, chunks):
                """fp16 chunks on DVE (4x mode) -> o16, sync ring."""
                rows = slice(t * P, (t + 1) * P)
                n = len(chunks)
                blk = blkp.tile([P, n * 1024], f16, tag="blk16")
                for s, c in enumerate(chunks):
                    nc.vector.tensor_scalar_mul(
                        out=blk[:, s * 1024 : (s + 1) * 1024],
                        in0=t5,
                        scalar1=scol(sc16, c),
                    )
                nc.sync.dma_start(
                    out=o16_d[rows, chunks[0] * 1024 : (chunks[0] + n) * 1024],
                    in_=blk,
                )

            def grp8(t, t5, sc16, chunks, eng):
                """fp8 chunks on DVE or ACT; returns the deferred
                sync-ring dma thunk so DMAs can be emitted in expected
                completion order (the sync FIFO dispatches strictly in
                emission order)."""
                rows = slice(t * P, (t + 1) * P)
                n = len(chunks)
                pool, tag = (blkp, "blk8") if eng == "v" else (bap, "ablk")
                blk = pool.tile([P, n * 1024], f8, tag=tag)
                for s, c in enumerate(chunks):
                    q = blk[:, s * 1024 : (s + 1) * 1024]
                    if eng == "a":
                        nc.scalar.mul(out=q, in_=t5, mul=scol(sc16, c))
                    else:
                        nc.vector.tensor_scalar_mul(
                            out=q, in0=t5, scalar1=scol(sc16, c)
                        )
                return lambda: nc.sync.dma_start(
                    out=o8_d[
                        rows,
                        chunks[0] * 1024 - C16 : (chunks[0] + n) * 1024 - C16,
                    ],
                    in_=blk,
                )

            def halves(lst):
                k = (len(lst) + 1) // 2
                return [lst[:k], lst[k:]] if lst[k:] else [lst[:k]]

            # all four cascades (incl. t5s, 8 KiB/partition) are built
            # upfront: from ~16us the DVE queue is pure chunk production
            # with no mid-run cascade lulls to starve the DMA stream
            casc_all = [cascade(t) for t in range(NTILES)]
            for t in range(NTILES):
                t5, sc16 = casc_all[t]
                par = t & 1
                last = t == NTILES - 1
                g16 = [[0], [1]] if last else [[0, 1]]
                for ch in g16:
                    grp16(t, t5, sc16, ch)
                ag = (
                    [[10, 11, 12], [13, 14], [15]]
                    if last
                    else halves(ACT8[par])
                )
                adma = [grp8(t, t5, sc16, ch, "a") for ch in ag]
                vg = (
                    [[2, 3, 4, 5], [6, 7], [8], [9]]
                    if last
                    else halves(DVE8[par])
                )
                vdma = [grp8(t, t5, sc16, ch, "v") for ch in vg]
                # sync-ring order = expected completion order; the last
                # tile tapers to single-chunk lumps so the final serial
                # drain after engines stop is minimal
                if last:
                    seq = [
                        adma[0], vdma[0], adma[1], vdma[1],
                        adma[2], vdma[2], vdma[3],
                    ]
                else:
                    seq = [vdma[0], adma[0], vdma[1], adma[1]]
                for d in seq:
                    d()
                if t == 1:
                    # softmax denominators (fused over all tiles); emitted
                    # mid-stream so they are not on the DVE queue's tail
                    s28 = cpool.tile([P, NX], f32)
                    nc.vector.tensor_reduce(
                        out=s28,
                        in_=e.rearrange("p (tf d) -> p tf d", d=D1),
                        axis=AX,
                        op=Alu.add,
                    )
                    nc.vector.tensor_reduce(
                        out=zbuf,
                        in_=s28.rearrange("p (t f) -> p t f", f=F),
                        axis=AX,
                        op=Alu.mult,
                    )
                    nc.sync.dma_start(out=z_d, in_=zbuf)
    nc.compile()
    return nc


def build_in_maps(x, cutpoints):
    inv_t = 1.0 / TEMPERATURE
    cp = np.sort(cutpoints.astype(np.float32), axis=1)  # (F, 3)
    b = np.cumsum(
        np.concatenate([np.zeros((F, 1), np.float32), -cp], axis=1), axis=1
    )  # (F, 4)
    wpat = np.arange(1.0, D1 + 1.0, dtype=np.float32) * inv_t  # 4 cols
    bflat = np.tile((b * inv_t).ravel(), NTILES).astype(np.float32)  # 112
    # x sharded: core k, partition p gets rows k*512 + {p, 128+p, 256+p, 384+p}
    xs = (
        x.reshape(NCORES, NTILES, P, F)
        .transpose(0, 2, 1, 3)
        .reshape(NCORES, P, NTILES * F)
    )
    in_maps = []
    for k in range(NCORES):
        xw = np.empty((P, XWC), dtype=np.float32)
        xw[:, 0:NX] = xs[k]
        xw[:, NX : NX + D1] = wpat
        xw[:, NX + D1 :] = bflat
        in_maps.append({"xw": xw})
    return in_maps


def postprocess(results):
    """fp16/fp8 unnormalized outputs + per-row Z -> normalized fp32."""
    parts = []
    for k in range(NCORES):
        z = np.asarray(results[k]["zp"])  # (P, NTILES), row t*128+p
        rec = (1.0 / z.T.reshape(ROWS, 1)).astype(np.float32)
        full = np.empty((ROWS, OUT), dtype=np.float32)
        full[:, 0:C16] = np.asarray(results[k]["o16"]).astype(np.float32)
        full[:, C16:] = np.asarray(results[k]["o8"]).astype(np.float32)
        full *= rec
        parts.append(full)
    return np.concatenate(parts, axis=0)


def kernel(x, cutpoints):
    from concourse import bass_utils

    if "nc" not in _cache:
        _cache["nc"] = _build_bass()
    nc = _cache["nc"]

    x = np.ascontiguousarray(np.asarray(x), dtype=np.float32)
    cutpoints = np.ascontiguousarray(np.asarray(cutpoints), dtype=np.float32)
    in_maps = build_in_maps(x, cutpoints)
    res = bass_utils.run_bass_kernel_spmd(nc, in_maps, list(range(NCORES))).results
    return postprocess(res)


# revision 30
# speedup vs baseline: 1.0014x; 1.0014x over previous
"""Trainium2 Bass kernel for soft decision-tree histogram binning.

Computes out[b, j] = prod_f softmax((x[b,f]*W + b_f)/T)[digit_f(j)]
for x (4096, 7), cutpoints (7, 3) -> out (4096, 4**7=16384) float32.

Strategy (data-parallel over batch, 8 cores x 512 rows, 4 tiles of 128):
  - mixed-precision output: chunks 0-1 of each tile row go out as fp16
    (produced by DVE in its 4x mode, 330 ns/chunk), chunks 2-15 as fp8e4
    (DVE 2x mode 662 ns/chunk, ACT 1131 ns/chunk; fp8 has no DVE
    packing on cayman). fp8 halves the HBM drain that bounded the
    all-fp16 kernel (59.2us). Unnormalized kron factors lie in (0,1]
    with the per-row max exactly 1.0 (exp(h-max)), which fp8e4 (RTNE,
    matches ml_dtypes e4m3) represents exactly: measured end-to-end rel
    err 8.0e-3 vs the 2e-2 gate.
  - softmax denominators are NOT applied on device: zp = prod_f sum_d e
    goes out as a tiny side output; the host folds 1/zp into the upcast.
  - h-chain and z-products are fused across all 4 tiles ([P,112] ops);
    t4 is fp16 so the t5 builders hit DVE 4x.
  - output DMAs: few and large (2-5 chunks each), all on the sync HWDGE
    ring, emitted per tile in expected completion order [fp16,
    DVE-half, ACT-half, DVE-half, ACT-half] (the sync FIFO dispatches
    strictly in emission order). The last tile ends with single-chunk
    groups so the final drain lump is small.
  - the four cascades (one fused 4D-AP op per level for t2/t3/t4/sc16
    across all tiles + 16 t5 builders) are built entirely upfront, so
    from ~16us both engine queues are pure chunk production with no
    mid-run lulls to starve the DMA stream (-1us vs per-tile cascades).
  - measured structural floor ~45-47us: ACT ends ~41.4 (busy 28.9),
    DVE ~40.8 (busy 30.7, 98% duty), then the last lump drains at the
    HBM-stack-shared DMA ceiling (~0.39 MiB/us/core; all 8 cores burst
    in lockstep) plus ~1.5us completion receipt + lean exit. Front is
    input-sem gated: input data lands ~6us but its completion sem fires
    ~9.2us (HBM receipt round-trip); a DVE probe op confirmed the
    engine itself is awake at ~7us.
  Tried and measurably worse:
  - per-chunk DMAs in completion order (+15us: per-DMA sync-sequencer
    overhead dominates),
  - ACT- or GpSimd-dispatched DMAs (the dispatching engine's compute
    stream stalls on ring backpressure; GpSimd SWDGE DIRECT2D also
    costs 6-8us descriptor gen per block),
  - GPSIMD as third fp8 producer (15-22us per chunk software dtype
    conversion + SBUF port contention stretches 2-port DVE ops ~3x),
  - more fp16 chunks (C16=5: bytes exceed the shared-stack drain
    ceiling, backpressure stalls both producers),
  - PE diag-matmul producer: dead end, DMA has no PSUM route on trn2
    so evacuation would cost the same DVE/ACT elem throughput anyway.
"""

import numpy as np

B = 4096
F = 7
D1 = 4  # D+1 bins per feature
OUT = D1**F  # 16384
NCORES = 8
ROWS = B // NCORES  # 512
P = 128
NTILES = ROWS // P  # 4
TEMPERATURE = 0.1

NX = NTILES * F  # 28 x columns
XWC = NX + D1 + NTILES * F * D1  # x (28) | W/T (4) | b/T replicated (112)

N16 = 2  # chunks 0..N16-1 are fp16 (DVE 4x); rest fp8
C16 = N16 * 1024
C8 = OUT - C16

# per-tile-parity fp8 chunk assignment
DVE8 = {0: [2, 3, 4, 5, 6, 7, 8, 9], 1: [2, 3, 4, 5, 6, 7, 8, 9]}
ACT8 = {0: [10, 11, 12, 13, 14, 15], 1: [10, 11, 12, 13, 14, 15]}

_cache = {}


def _build_bass():
    import concourse.bacc as bacc
    import concourse.tile as tile
    from concourse import mybir

    f32 = mybir.dt.float32
    f16 = mybir.dt.float16
    f8 = mybir.dt.float8e4
    Alu = mybir.AluOpType
    Act = mybir.ActivationFunctionType
    AX = mybir.AxisListType.X

    from concourse.vector_clock import ScopedClock

    class LeanTileContext(tile.TileContext):
        """TileContext with a minimal kernel exit: keep the sync-engine
        drain that waits for all outstanding work, skip the two
        all-engine barriers and the semaphore recycle loop."""

        def _drain_and_barrier(self, tick_clock, wait_clock):
            drain_inst = self.nc.sync.drain()
            wait_clock.add_sem_waits(
                drain_inst.ins, ScopedClock({None: tick_clock.global_clock})
            )
            popped = self.nc._tile_sem_poison_stack.pop()
            assert popped is self._sem_poison

    nc = bacc.Bacc("TRN2", target_bir_lowering=False, debug=False)

    xw_d = nc.dram_tensor("xw", [P, XWC], f32, kind="ExternalInput").ap()
    o16_d = nc.dram_tensor("o16", [ROWS, C16], f16, kind="ExternalOutput").ap()
    o8_d = nc.dram_tensor("o8", [ROWS, C8], f8, kind="ExternalOutput").ap()
    z_d = nc.dram_tensor("zp", [P, NTILES], f32, kind="ExternalOutput").ap()

    with LeanTileContext(nc) as tc:
        with (
            tc.tile_pool(name="const", bufs=1) as cpool,
            tc.tile_pool(name="small", bufs=3) as sp,
            tc.tile_pool(name="mid", bufs=5) as mp,
            tc.tile_pool(name="blk", bufs=8) as blkp,
            tc.tile_pool(name="blka", bufs=6) as bap,
        ):
            xw = cpool.tile([P, XWC], f32)
            nc.sync.dma_start(out=xw, in_=xw_d)
            w4 = xw[:, NX : NX + D1][:, None, :].broadcast_to((P, NX, D1))
            ball = xw[:, NX + D1 :].rearrange("p (tf d) -> p tf d", d=D1)
            zbuf = cpool.tile([P, NTILES], f32)

            # fused h-chain over all 4 tiles: h[(t f), d] = x*(W/T) + b/T
            h = cpool.tile([P, NX * D1], f32)
            h3 = h.rearrange("p (tf d) -> p tf d", d=D1)
            xb = xw[:, 0:NX][:, :, None].broadcast_to((P, NX, D1))
            nc.vector.tensor_tensor(out=h3, in0=xb, in1=w4, op=Alu.mult)
            nc.vector.tensor_tensor(out=h3, in0=h3, in1=ball, op=Alu.add)
            m28 = cpool.tile([P, NX], f32)
            nc.vector.tensor_reduce(out=m28, in_=h3, axis=AX, op=Alu.max)
            mb = m28[:, :, None].broadcast_to((P, NX, D1))
            nc.vector.tensor_tensor(out=h3, in0=h3, in1=mb, op=Alu.subtract)
            e = cpool.tile([P, NX * D1], f32)
            nc.scalar.activation(out=e, in_=h, func=Act.Exp, scale=1.0)

            # cascade first halves (t2/t3/t4/sc16) fused across all 4
            # tiles: the tiles share the same 128 partitions, so one 4D-AP
            # op per level replaces four, amortizing DVE op-init overhead
            et4 = e.rearrange("p (t c) -> p t c", c=28)  # [P, 4, 28]
            t2a = cpool.tile([P, NTILES * 16], f32)
            nc.vector.tensor_tensor(
                out=t2a.rearrange("p (t a b) -> p t a b", a=D1, b=D1),
                in0=et4[:, :, 20:24][:, :, :, None].broadcast_to(
                    (P, NTILES, D1, D1)
                ),
                in1=et4[:, :, 24:28][:, :, None, :].broadcast_to(
                    (P, NTILES, D1, D1)
                ),
                op=Alu.mult,
            )
            t3a = cpool.tile([P, NTILES * 64], f32)
            nc.vector.tensor_tensor(
                out=t3a.rearrange("p (t a b) -> p t a b", a=D1, b=16),
                in0=et4[:, :, 16:20][:, :, :, None].broadcast_to(
                    (P, NTILES, D1, 16)
                ),
                in1=t2a.rearrange("p (t b) -> p t b", b=16)[
                    :, :, None, :
                ].broadcast_to((P, NTILES, D1, 16)),
                op=Alu.mult,
            )
            t4a = cpool.tile([P, NTILES * 256], f16)
            nc.vector.tensor_tensor(
                out=t4a.rearrange("p (t a b) -> p t a b", a=D1, b=64),
                in0=et4[:, :, 12:16][:, :, :, None].broadcast_to(
                    (P, NTILES, D1, 64)
                ),
                in1=t3a.rearrange("p (t b) -> p t b", b=64)[
                    :, :, None, :
                ].broadcast_to((P, NTILES, D1, 64)),
                op=Alu.mult,
            )
            sca = cpool.tile([P, NTILES * 16], f32)
            nc.vector.tensor_tensor(
                out=sca.rearrange("p (t a b) -> p t a b", a=D1, b=D1),
                in0=et4[:, :, 4:8][:, :, :, None].broadcast_to(
                    (P, NTILES, D1, D1)
                ),
                in1=et4[:, :, 0:4][:, :, None, :].broadcast_to(
                    (P, NTILES, D1, D1)
                ),
                op=Alu.mult,
            )

            def casc_b(t):
                """Cascade second half: the four t5 builders (DVE 4x)."""
                et = e[:, t * 28 : (t + 1) * 28]
                t4 = t4a[:, t * 256 : (t + 1) * 256]
                t5 = mp.tile([P, 1024], f16, tag="t5")
                for d in range(D1):
                    nc.vector.tensor_scalar_mul(
                        out=t5[:, d * 256 : (d + 1) * 256],
                        in0=t4,
                        scalar1=et[:, 8 + d : 9 + d],
                    )
                return t5

            def cascade(t):
                return casc_b(t), sca[:, t * 16 : (t + 1) * 16]

            def scol(sc16, c):
                d0, d1 = c // D1, c % D1
                return sc16[:, d1 * D1 + d0 : d1 * D1 + d0 + 1]

            def grp16(t, t5, sc16, chunks):
                """fp16 chunks on DVE (4x mode) -> o16, sync ring."""
                rows = slice(t * P, (t + 1) * P)
                n = len(chunks)
                blk = blkp.tile([P, n * 1024], f16, tag="blk16")
                for s, c in enumerate(chunks):
                    nc.vector.tensor_scalar_mul(
                        out=blk[:, s * 1024 : (s + 1) * 1024],
                        in0=t5,
                        scalar1=scol(sc16, c),
                    )
                nc.sync.dma_start(
                    out=o16_d[rows, chunks[0] * 1024 : (chunks[0] + n) * 1024],
                    in_=blk,
                )

            def grp8(t, t5, sc16, chunks, eng):
                """fp8 chunks on DVE or ACT; returns the deferred
                sync-ring dma thunk so DMAs can be emitted in expected
                completion order (the sync FIFO dispatches strictly in
                emission order)."""
                rows = slice(t * P, (t + 1) * P)
                n = len(chunks)
                pool, tag = (blkp, "blk8") if eng == "v" else (bap, "ablk")
                blk = pool.tile([P, n * 1024], f8, tag=tag)
                for s, c in enumerate(chunks):
                    q = blk[:, s * 1024 : (s + 1) * 1024]
                    if eng == "a":
                        nc.scalar.mul(out=q, in_=t5, mul=scol(sc16, c))
                    else:
                        nc.vector.tensor_scalar_mul(
                            out=q, in0=t5, scalar1=scol(sc16, c)
                        )
                return lambda: nc.sync.dma_start(
                    out=o8_d[
                        rows,
                        chunks[0] * 1024 - C16 : (chunks[0] + n) * 1024 - C16,
                    ],
                    in_=blk,
                )

            def halves(lst):
                k = (len(lst) + 1) // 2
                return [lst[:k], lst[k:]] if lst[k:] else [lst[:k]]

            # all four cascades (incl. t5s, 8 KiB/partition) are built
            # upfront: from ~16us the DVE queue is pure chunk production
            # with no mid-run cascade lulls to starve the DMA stream
            casc_all = [cascade(t) for t in range(NTILES)]
            for t in range(NTILES):
                t5, sc16 = casc_all[t]
                par = t & 1
                last = t == NTILES - 1
                g16 = [[0], [1]] if last else [[0, 1]]
                for ch in g16:
                    grp16(t, t5, sc16, ch)
                ag = (
                    [[10, 11, 12], [13, 14], [15]]
                    if last
                    else halves(ACT8[par])
                )
                adma = [grp8(t, t5, sc16, ch, "a") for ch in ag]
                vg = (
                    [[2, 3, 4, 5], [6, 7], [8], [9]]
                    if last
                    else halves(DVE8[par])
                )
                vdma = [grp8(t, t5, sc16, ch, "v") for ch in vg]
                # sync-ring order = expected completion order; the last
                # tile tapers to single-chunk lumps so the final serial
                # drain after engines stop is minimal
                if last:
                    seq = [
                        adma[0], vdma[0], adma[1], vdma[1],
                        adma[2], vdma[2], vdma[3],
                    ]
                else:
                    seq = [vdma[0], adma[0], vdma[1], adma[1]]
                for d in seq:
                    d()
                if t == 1:
                    # softmax denominators (fused over all tiles); emitted
                    # mid-stream so they are not on the DVE queue's tail
                    s28 = cpool.tile([P, NX], f32)
                    nc.vector.tensor_reduce(
                        out=s28,
                        in_=e.rearrange("p (tf d) -> p tf d", d=D1),
                        axis=AX,
                        op=Alu.add,
                    )
                    nc.vector.tensor_reduce(
                        out=zbuf,
                        in_=s28.rearrange("p (t f) -> p t f", f=F),
                        axis=AX,
                        op=Alu.mult,
                    )
                    nc.sync.dma_start(out=z_d, in_=zbuf)
    nc.compile()
    return nc


def build_in_maps(x, cutpoints):
    inv_t = 1.0 / TEMPERATURE
    cp = np.sort(cutpoints.astype(np.float32), axis=1)  # (F, 3)
    b = np.cumsum(
        np.concatenate([np.zeros((F, 1), np.float32), -cp], axis=1), axis=1
    )  # (F, 4)
    wpat = np.arange(1.0, D1 + 1.0, dtype=np.float32) * inv_t  # 4 cols
    bflat = np.tile((b * inv_t).ravel(), NTILES).astype(np.float32)  # 112
    # x sharded: core k, partition p gets rows k*512 + {p, 128+p, 256+p, 384+p}
    xs = (
        x.reshape(NCORES, NTILES, P, F)
        .transpose(0, 2, 1, 3)
        .reshape(NCORES, P, NTILES * F)
    )
    in_maps = []
    for k in range(NCORES):
        xw = np.empty((P, XWC), dtype=np.float32)
        xw[:, 0:NX] = xs[k]
        xw[:, NX : NX + D1] = wpat
        xw[:, NX + D1 :] = bflat
        in_maps.append({"xw": xw})
    return in_maps


def postprocess(results):
    """fp16/fp8 unnormalized outputs + per-row Z -> normalized fp32."""
    parts = []
    for k in range(NCORES):
        z = np.asarray(results[k]["zp"])  # (P, NTILES), row t*128+p
        rec = (1.0 / z.T.reshape(ROWS, 1)).astype(np.float32)
        full = np.empty((ROWS, OUT), dtype=np.float32)
        full[:, 0:C16] = np.asarray(results[k]["o16"]).astype(np.float32)
        full[:, C16:] = np.asarray(results[k]["o8"]).astype(np.float32)
        full *= rec
        parts.append(full)
    return np.concatenate(parts, axis=0)


def kernel(x, cutpoints):
    from concourse import bass_utils

    if "nc" not in _cache:
        _cache["nc"] = _build_bass()
    nc = _cache["nc"]

    x = np.ascontiguousarray(np.asarray(x), dtype=np.float32)
    cutpoints = np.ascontiguousarray(np.asarray(cutpoints), dtype=np.float32)
    in_maps = build_in_maps(x, cutpoints)
    res = bass_utils.run_bass_kernel_spmd(nc, in_maps, list(range(NCORES))).results
    return postprocess(res)
